# revision 67
# baseline (speedup 1.0000x reference)
"""Trainium2 Bass kernel for an AttentionBlock (1x1-conv QKV -> full spatial
attention -> 1x1-conv out + skip), data-parallel over batch across 8 cores.

Per-core problem (one batch element):
  x      [512, 4096]  (C, N) with N = 64*64
  qkv    = w_in @ x + b_in       -> q,k,v each [64, 4096]
  S^T    = k^T (q*scale)         computed as [keys, queries] tiles
  U      = exp(S^T)              (no max subtraction; |S| < ~1.5 for this data)
  O^T    = v U (+ ones row -> softmax denominators), normalized per query
  y      = w_out @ O + b_out + x

v4 design (on top of v3):
  - startup: x loads as 8 fat DMAs ([128, 2048], 4KB/partition elements)
    spread over FOUR queues (Sync HWDGE, Scalar HWDGE, GpSimd SWDGE q0/q1)
    so block 0 lands in ~3us instead of ~20us; weights ride ahead on the
    sync/scalar queues; gpsimd memsets are emitted after the DMA issues.
  - attention-V runs TWO pairs behind the score matmuls (depth-2 pipeline,
    u ring of 6) so exp latency never stalls the PE FIFO.
  - the first score pair of super-chunk n+1 is emitted (with its exp)
    before the last attnv of super-chunk n, removing the cross-SC PE
    bubble; the softmax-denominator chain starts per-query-chunk inside
    the final attnv (recip reads PSUM directly, no staging copy).
  - the last super-chunk runs as two single-chunk phases (all of qa's
    pairs, then all of qb's); qa's normalize/out-projection overlap qb's
    attention, shrinking the serial tail from ~14us to ~4us.
  - scores use 64x128 row-tiled matmul pairs (tiles T0/T8) as in v3; exp
    splits between ScalarE (exact) and VectorE (Schraudolph-to-fp8);
    attnv uses fp8 DoubleRow with virtual K=256.
"""

import numpy as np
import ml_dtypes

from concourse import bacc, tile, mybir
from concourse import bass_utils
from concourse.bass import ds, ts
from concourse.masks import make_identity

F32 = mybir.dt.float32
BF16 = mybir.dt.bfloat16
I16 = mybir.dt.int16
F8 = mybir.dt.float8e4
I8 = mybir.dt.int8
DR = mybir.MatmulPerfMode.DoubleRow
EXP = mybir.ActivationFunctionType.Exp
IDENT = mybir.ActivationFunctionType.Identity
MULT = mybir.AluOpType.mult
ADD = mybir.AluOpType.add

B = 8
C = 512
HID = 64
N = 4096
NMT = N // 128      # 32 key tiles
QC = 512            # query chunk (PSUM bank width in f32)
PAIRS = NMT // 2    # 16 key-tile pairs
NSC = 4             # super-chunks of 2 query chunks

# Schraudolph exp->fp8e4m3 bit trick: i8 = s*A + Bc, bitcast to fp8
SCH_A = 8.0 / float(np.log(2.0))
SCH_B = 56.0 - 0.46

# (p, j) pairs whose exp runs on VectorE (12 of 32 per super-chunk; ScalarE
# also carries the qq-bias and the per-super-chunk O staging copies)
VEC_PJ = frozenset((p, j) for p in range(PAIRS) for j in range(2)
                   if (2 * p + j) % 32 in (1, 4, 7, 10, 13, 14, 16, 19, 22, 25,
                                           28, 31))


def build_bass(stage=4):
    nc = bacc.Bacc(
        "TRN2",
        target_bir_lowering=False,
        debug=False,
        enable_asserts=False,
        num_devices=B,
    )
    x = nc.dram_tensor("x", [C, N], BF16, kind="ExternalInput").ap()
    wkvT = nc.dram_tensor("wkvT", [C, 128], BF16, kind="ExternalInput").ap()
    wqqT = nc.dram_tensor("wqqT", [C, 128], BF16, kind="ExternalInput").ap()
    bkv = nc.dram_tensor("bkv", [128, 1], F32, kind="ExternalInput").ap()
    bqq = nc.dram_tensor("bqq", [128, 1], F32, kind="ExternalInput").ap()
    woT = nc.dram_tensor("woT", [HID, C], BF16, kind="ExternalInput").ap()
    y = nc.dram_tensor("y", [C, N], BF16, kind="ExternalOutput").ap()

    xr = x.rearrange("(a p) n -> p a n", p=128)   # [128, 4, N] channel = a*128+p
    yr = y.rearrange("(a p) n -> p a n", p=128)

    with tile.TileContext(nc) as tc:
        with (
            nc.allow_low_precision(reason="bf16/approx-exp attention is intended"),
            tc.tile_pool(name="const", bufs=1) as cpool,
            tc.tile_pool(name="big", bufs=1) as bigpool,
            tc.tile_pool(name="u", bufs=6) as upool,
            tc.tile_pool(name="work", bufs=2) as wpool,
            tc.tile_pool(name="yout", bufs=6) as ypool,
            tc.tile_pool(name="psum", bufs=3, space="PSUM") as pp,
        ):
            # ---- persistent tensors (allocate before constants so DMAs can
            # be emitted first) ----
            xb = bigpool.tile([128, 4, N], BF16)      # x (already bf16 in DRAM)
            vk = bigpool.tile([128, N], BF16)         # rows 0:64 v, 64:128 k
            klo = bigpool.tile([128, N], BF16)        # rows 0:64 = k
            qq = bigpool.tile([128, N], BF16)         # q*scale on both halves
            vt = bigpool.tile([128, NMT, 80], F8)     # v^T tiles + ones col 64
            Ob = bigpool.tile([128, N], BF16)         # rows 0:64 normalized O

            wkv = cpool.tile([128, 4, 128], BF16)
            wqq = cpool.tile([128, 4, 128], BF16)
            bkv_sb = cpool.tile([128, 1], F32)
            bqq_sb = cpool.tile([128, 1], F32)
            # w_out^T on both partition halves: the out-projection runs as
            # two concurrent row-tiled K=64 matmuls (the b_out bias is
            # folded into x on the host; QKV biases are compensated there)
            wo2 = cpool.tile([128, C], BF16)

            # critical DMA wave: x block 0 (one chunk per queue; SWDGE takes
            # two) plus the qkv weights; block-0 chunks lead each queue so
            # the first projection can start as early as possible
            def emit_x(b):
                cols = ts(b, 1024)
                nc.sync.dma_start(xb[:, 0, cols], xr[:, 0, cols])
                nc.scalar.dma_start(xb[:, 1, cols], xr[:, 1, cols])
                nc.gpsimd.dma_start(xb[:, 2, cols], xr[:, 2, cols])
                nc.gpsimd.dma_start(xb[:, 3, cols], xr[:, 3, cols])

            nc.scalar.dma_start(wkv[:, :, :], wkvT.rearrange("(a p) m -> p a m", p=128))
            # identity first on gpsimd: it feeds the PE warm-up matmuls
            ident_f = cpool.tile([64, 64], F32)
            make_identity(nc, ident_f[:, :])
            emit_x(0)
            nc.sync.dma_start(bkv_sb[:, :], bkv)
            nc.scalar.dma_start(wqq[:, :, :], wqqT.rearrange("(a p) m -> p a m", p=128))
            nc.scalar.dma_start(bqq_sb[:, :], bqq)
            nc.sync.dma_start(wo2[0:HID, :], woT)
            nc.sync.dma_start(wo2[HID:128, :], woT)

            # ---- constants (gpsimd/vector work after the DMA issues) ----
            ident = cpool.tile([64, 64], BF16)
            nc.vector.tensor_copy(ident[:, :], ident_f[:, :])
            wrm = cpool.tile([64, 512], BF16)
            nc.gpsimd.memset(wrm[:, :], 1.0)
            ones_f = cpool.tile([128, NMT], F32)
            nc.gpsimd.memset(ones_f[:, :], 1.0)

            nc.vector.tensor_copy(vt[:, :, HID], ones_f[:, 0:NMT])

            # HAM warm-up: a dense stream of N=512 matmuls (identity weights
            # against a memset tile -- the product is never read) keeps the
            # PE activity window busy while x streams in, so the projection
            # and score matmuls run at 2.4 GHz from the start.
            ps_w = pp.tile([64, 512], F32, tag="pair", name="ps_warm")
            for r in range(20):
                nc.tensor.matmul(
                    ps_w[:, :], ident[:, :], wrm[:, :],
                    start=(r == 0), stop=(r == 19),
                )

            def emit_vk_mms(b):
                nblk = ts(b, 1024)
                ps_vk = pp.tile([128, 1024], F32, tag="pair", name=f"psvk_{b}")
                for c2 in range(0, 1024, 512):
                    cols = ds(b * 1024 + c2, 512)
                    for kc in range(4):
                        nc.tensor.matmul(
                            ps_vk[:, c2:c2 + 512], wkv[:, kc, :], xb[:, kc, cols],
                            start=(kc == 0), stop=(kc == 3),
                        )
                return ps_vk

            def emit_vk_post(b, ps_vk):
                """bias add, k-low swap, v^T tiles for block b"""
                nblk = ts(b, 1024)
                nc.vector.tensor_scalar_add(vk[:, nblk], ps_vk[:, :], bkv_sb[:, 0:1])
                nc.sync.dma_start(klo[0:64, nblk], vk[64:128, nblk])
                for t2 in range(4):
                    mt = b * 8 + 2 * t2
                    ps_t = pp.tile([128, 128], BF16, tag="pair", bufs=3,
                                   name=f"pst_{mt}")
                    nc.tensor.transpose(ps_t[:, 0:64], vk[0:64, ts(mt, 128)],
                                        ident[:, :])
                    nc.tensor.transpose(ps_t[:, 64:128], vk[0:64, ts(mt + 1, 128)],
                                        ident[:, :])
                    src = ps_t.rearrange("p (two f) -> p two f", two=2)
                    if t2 % 2 == 0:
                        nc.scalar.copy(vt[:, mt:mt + 2, 0:HID], src[:, :, :])
                    else:
                        nc.vector.tensor_copy(vt[:, mt:mt + 2, 0:HID], src[:, :, :])

            def emit_proj_vk(b):
                emit_vk_post(b, emit_vk_mms(b))

            def emit_proj_qq(b):
                nblk = ts(b, 1024)
                ps_qq = pp.tile([128, 1024], F32, tag="pair", name=f"psqq_{b}")
                for c2 in range(0, 1024, 512):
                    cols = ds(b * 1024 + c2, 512)
                    for kc in range(4):
                        nc.tensor.matmul(
                            ps_qq[:, c2:c2 + 512], wqq[:, kc, :], xb[:, kc, cols],
                            start=(kc == 0), stop=(kc == 3),
                        )
                nc.scalar.activation(qq[:, nblk], ps_qq[:, :], IDENT,
                                     bias=bqq_sb[:, 0:1])

            # block 0: vk and qq matmuls back-to-back keep the PE stream
            # dense (no HAM dip); bias/transposes follow, then wave 2 of x
            ps_vk0 = emit_vk_mms(0)
            emit_proj_qq(0)
            emit_vk_post(0, ps_vk0)
            emit_x(1)

            ps_o_tiles = {}
            rb_tiles = {}
            otmp_tiles = {}

            def score_pair(qc, p):
                """scores for key pair p against query chunk qc -> PSUM tile"""
                qblk = ds(qc * QC, QC)
                mt0, mt1 = 2 * p, 2 * p + 1
                pr = pp.tile([128, 1024], F32, tag="pair", name=f"ps_{qc}_{p}")
                nc.tensor.matmul(
                    pr[:, 0:512], klo[0:64, ts(mt0, 128)], qq[0:64, qblk],
                    start=True, stop=True, tile_position=(0, 0),
                )
                nc.tensor.matmul(
                    pr[:, 512:1024], vk[64:128, ts(mt1, 128)], qq[64:128, qblk],
                    start=True, stop=True, tile_position=(64, 0),
                )
                return pr

            def exp_pair(qc, p, j, pr):
                u = upool.tile([128, 1024], F8, tag="u", name=f"u_{qc}_{p}")
                if (p, j) in VEC_PJ:
                    nc.vector.tensor_scalar(
                        u.bitcast(I8)[:, :], pr[:, :], SCH_A, SCH_B, MULT, ADD,
                    )
                else:
                    nc.scalar.activation(u[:, :], pr[:, :], EXP)
                return u

            def attnv_j(p, jslot, u, start, stop):
                # fp8 DoubleRow: one matmul contracts both key tiles of the
                # pair (virtual K=256); lhsT [128,2,65], rhs [128,2,512]
                nc.tensor.matmul(
                    ps_o_tiles_j[jslot][0:HID + 1, :],
                    vt[:, 2 * p:2 * p + 2, 0:HID + 1],
                    u.rearrange("p (two f) -> p two f", two=2),
                    start=start, stop=stop,
                    perf_mode=DR,
                )

            def emit_d_hops(qc, early_copy=True):
                """one [65,512] DVE copy stages O rows + denominator row to
                SBUF (same cost as copying the denominator row alone), so
                the PSUM accumulator slot frees after a single op and the
                reciprocal chain runs entirely off the critical path."""
                ps_o = ps_o_tiles[qc]
                dsb = wpool.tile([1, QC], F32, tag="d", name=f"dsb_{qc}")
                nc.vector.tensor_copy(dsb[:, :], ps_o[HID:HID + 1, :])
                if early_copy:
                    ot = wpool.tile([HID, QC], F32, tag="ot", name=f"ot_{qc}")
                    nc.scalar.copy(ot[:, :], ps_o[0:HID, :])
                    otmp_tiles[qc] = ot
                rsb = wpool.tile([1, QC], F32, tag="r", name=f"rsb_{qc}")
                nc.vector.reciprocal_approx_fast(rsb[:, :], dsb[:, :])
                rb = wpool.tile([HID, QC], F32, tag="rb", name=f"rb_{qc}")
                nc.gpsimd.partition_broadcast(rb[:, :], rsb[:, :], channels=HID)
                rb_tiles[qc] = rb

            def emit_norm(qc):
                qblk = ds(qc * QC, QC)
                ps_o = ps_o_tiles.pop(qc)
                if qc in otmp_tiles:
                    src = otmp_tiles.pop(qc)[:, :]
                else:
                    src = ps_o[0:HID, :]
                nc.vector.tensor_mul(Ob[0:HID, qblk], src,
                                     rb_tiles.pop(qc)[:, :])
                # duplicate to the upper partition half for the row-tiled
                # out-projection pair (scalar HWDGE queue: idle in steady
                # state, so the copy lands well before the h64 matmuls)
                nc.scalar.dma_start(Ob[HID:128, qblk], Ob[0:HID, qblk])

            def emit_outproj(qc, ocp):
                """out-projection for channel groups 2*ocp, 2*ocp+1 as two
                concurrent row-tiled K=64 matmuls"""
                qblk = ds(qc * QC, QC)
                for half in (0, 1):
                    oc = 2 * ocp + half
                    ps_y = pp.tile([128, QC], F32, tag="pair", bufs=3,
                                   name=f"psy_{qc}_{oc}")
                    if half == 0:
                        nc.tensor.matmul(
                            ps_y[:, :], wo2[0:HID, ts(oc, 128)], Ob[0:HID, qblk],
                            start=True, stop=True, tile_position=(0, 0),
                        )
                    else:
                        nc.tensor.matmul(
                            ps_y[:, :], wo2[HID:128, ts(oc, 128)],
                            Ob[HID:128, qblk],
                            start=True, stop=True, tile_position=(64, 0),
                        )
                    y_sb = ypool.tile([128, QC], BF16, tag="ysb",
                                      name=f"ysb_{qc}_{oc}")
                    nc.vector.tensor_add(y_sb[:, :], ps_y[:, :], xb[:, oc, qblk])
                    nc.sync.dma_start(yr[:, oc, qblk], y_sb[:, :])

            ps_o_tiles_j = {}

            def super_chunk(sc):
                qa, qb = 2 * sc, 2 * sc + 1
                for j, qc in enumerate((qa, qb)):
                    t = pp.tile([128, QC], F32, tag="o", bufs=2, name=f"pso_{qc}")
                    ps_o_tiles_j[j] = t
                    ps_o_tiles[qc] = t
                us = {}
                for p in range(PAIRS):
                    if sc == 0 and p in (4, 8, 12):
                        emit_proj_vk(p // 4)
                        if p < 12:
                            emit_x(p // 4 + 1)
                    if sc > 0 and p == 1:
                        emit_norm(qa - 2)
                        emit_norm(qb - 2)
                    for j, qc in enumerate((qa, qb)):
                        pr = score_pair(qc, p)
                        us[(p, j)] = exp_pair(qc, p, j, pr)
                    # attnv runs two pairs behind so exp latency stays off
                    # the PE critical path; out-proj of the previous
                    # super-chunk rides in the same untiled-mode window.
                    if p >= 2:
                        for j in (0, 1):
                            attnv_j(p - 2, j, us.pop((p - 2, j)),
                                    start=(p - 2 == 0), stop=False)
                        if sc > 0 and p in (5, 7, 9, 11):
                            emit_outproj(qa - 2 if p <= 7 else qb - 2,
                                         (p // 2) % 2)
                    if sc < NSC - 1 and p == 12:
                        emit_proj_qq(sc + 1)
                for j in (0, 1):
                    attnv_j(PAIRS - 2, j, us.pop((PAIRS - 2, j)),
                            start=False, stop=False)
                for j, qc in enumerate((qa, qb)):
                    attnv_j(PAIRS - 1, j, us.pop((PAIRS - 1, j)),
                            start=False, stop=True)
                    emit_d_hops(qc, early_copy=True)

            def super_chunk_split(sc):
                """last super-chunk: qa fully, then qb; qa's normalize and
                out-projection overlap qb's attention phase."""
                qa, qb = 2 * sc, 2 * sc + 1
                for j, qc in enumerate((qa, qb)):
                    t = pp.tile([128, QC], F32, tag="o", bufs=2, name=f"pso_{qc}")
                    ps_o_tiles_j[j] = t
                    ps_o_tiles[qc] = t
                us = {}
                # ---- phase 0: all of qa's pairs ----
                for p in range(PAIRS):
                    if p == 1:
                        emit_norm(qa - 2)
                        emit_norm(qb - 2)
                    pr = score_pair(qa, p)
                    us[p] = exp_pair(qa, p, 0, pr)
                    if p >= 2:
                        attnv_j(p - 2, 0, us.pop(p - 2),
                                start=(p - 2 == 0), stop=False)
                        if p in (5, 7, 9, 11):
                            emit_outproj(qa - 2 if p <= 7 else qb - 2,
                                         (p // 2) % 2)
                # transition: qb's first four score pairs (full u-ring
                # depth) cover qa's last attnv exp latency and the
                # denominator chain
                usb = {}
                pr = score_pair(qb, 0)
                usb[0] = exp_pair(qb, 0, 1, pr)
                attnv_j(PAIRS - 2, 0, us.pop(PAIRS - 2), start=False, stop=False)
                pr = score_pair(qb, 1)
                usb[1] = exp_pair(qb, 1, 1, pr)
                attnv_j(PAIRS - 1, 0, us.pop(PAIRS - 1), start=False, stop=True)
                emit_d_hops(qa)
                pr = score_pair(qb, 2)
                usb[2] = exp_pair(qb, 2, 1, pr)
                attnv_j(0, 1, usb.pop(0), start=True, stop=False)
                pr = score_pair(qb, 3)
                usb[3] = exp_pair(qb, 3, 1, pr)
                attnv_j(1, 1, usb.pop(1), start=False, stop=False)
                emit_norm(qa)
                # ---- phase 1: qb's remaining pairs; qa's out-projection
                # rides inside ----
                for p in range(4, PAIRS):
                    pr = score_pair(qb, p)
                    usb[p] = exp_pair(qb, p, 1, pr)
                    attnv_j(p - 2, 1, usb.pop(p - 2), start=False, stop=False)
                    if p in (5, 13):
                        emit_outproj(qa, {5: 0, 13: 1}[p])
                attnv_j(PAIRS - 2, 1, usb.pop(PAIRS - 2), start=False, stop=False)
                attnv_j(PAIRS - 1, 1, usb.pop(PAIRS - 1), start=False, stop=True)
                emit_d_hops(qb)
                emit_norm(qb)
                for ocp in range(2):
                    emit_outproj(qb, ocp)

            for sc in range(NSC - 1):
                super_chunk(sc)
            super_chunk_split(NSC - 1)

    nc.compile()
    return nc


_NC = None
_NC_STAGE = None


def _get_nc(stage=4):
    global _NC, _NC_STAGE
    if _NC is None or _NC_STAGE != stage:
        _NC = build_bass(stage)
        _NC_STAGE = stage
    return _NC


def make_in_maps(x, w_in, b_in, w_out, b_out):
    scale = 1.0 / np.sqrt(np.float32(HID))
    w = np.asarray(w_in, np.float32)
    b = np.asarray(b_in, np.float32)
    bo = np.asarray(b_out, np.float32)                     # [512]
    # fold b_out into x (the skip connection carries it) and compensate
    # the qkv biases: q = Wq (x+bo) + (bq - Wq bo) == Wq x + bq, exactly
    bq = (b[0:HID] - w[0:HID] @ bo) * scale
    bk = b[HID:2 * HID] - w[HID:2 * HID] @ bo
    bv = b[2 * HID:] - w[2 * HID:] @ bo
    wq = np.ascontiguousarray(w[0:HID].T) * scale          # [512, 64]
    wk = np.ascontiguousarray(w[HID:2 * HID].T)
    wv = np.ascontiguousarray(w[2 * HID:3 * HID].T)
    wkvT = np.concatenate([wv, wk], axis=1)                # [512, 128] = [v|k]
    wqqT = np.concatenate([wq, wq], axis=1)
    bkv = np.concatenate([bv, bk]).reshape(128, 1)
    bqq = np.concatenate([bq, bq]).reshape(128, 1)
    woT = np.ascontiguousarray(np.asarray(w_out, np.float32).T)  # [64, 512]
    x = np.asarray(x, np.float32) + bo[None, :, None, None]
    return [
        {
            "x": np.ascontiguousarray(
                x[bb].reshape(C, N).astype(ml_dtypes.bfloat16)),
            "wkvT": np.ascontiguousarray(wkvT.astype(ml_dtypes.bfloat16)),
            "wqqT": np.ascontiguousarray(wqqT.astype(ml_dtypes.bfloat16)),
            "bkv": np.ascontiguousarray(bkv, np.float32),
            "bqq": np.ascontiguousarray(bqq, np.float32),
            "woT": np.ascontiguousarray(woT.astype(ml_dtypes.bfloat16)),
        }
        for bb in range(B)
    ]


def kernel(x, w_in, b_in, w_out, b_out):
    nc = _get_nc()
    in_maps = make_in_maps(x, w_in, b_in, w_out, b_out)
    res = bass_utils.run_bass_kernel_spmd(nc, in_maps, core_ids=list(range(B)))
    H = int(np.sqrt(N))
    out = np.stack([
        np.asarray(res.results[bb]["y"]).astype(np.float32).reshape(C, H, H)
        for bb in range(B)
    ])
    return out


# revision 70
# speedup vs baseline: 1.1718x; 1.1718x over previous
"""Trainium2 Bass kernel for an AttentionBlock (1x1-conv QKV -> full spatial
attention -> 1x1-conv out + skip), data-parallel over batch across 8 cores.

Per-core problem (one batch element):
  x      [512, 4096]  (C, N) with N = 64*64
  qkv    = w_in @ x + b_in       -> q,k,v each [64, 4096]
  S^T    = k^T (q*scale)         computed as [keys, queries] tiles
  U      = exp(S^T)              (no max subtraction; |S| < ~1.5 for this data)
  O^T    = v U (+ ones row -> softmax denominators), normalized per query
  y      = w_out @ O + b_out + x

v4 design (on top of v3):
  - startup: x loads as 8 fat DMAs ([128, 2048], 4KB/partition elements)
    spread over FOUR queues (Sync HWDGE, Scalar HWDGE, GpSimd SWDGE q0/q1)
    so block 0 lands in ~3us instead of ~20us; weights ride ahead on the
    sync/scalar queues; gpsimd memsets are emitted after the DMA issues.
  - attention-V runs TWO pairs behind the score matmuls (depth-2 pipeline,
    u ring of 6) so exp latency never stalls the PE FIFO.
  - the first score pair of super-chunk n+1 is emitted (with its exp)
    before the last attnv of super-chunk n, removing the cross-SC PE
    bubble; the softmax-denominator chain starts per-query-chunk inside
    the final attnv (recip reads PSUM directly, no staging copy).
  - the last super-chunk runs as two single-chunk phases (all of qa's
    pairs, then all of qb's); qa's normalize/out-projection overlap qb's
    attention, shrinking the serial tail from ~14us to ~4us.
  - scores use 64x128 row-tiled matmul pairs (tiles T0/T8) as in v3; exp
    splits between ScalarE (exact) and VectorE (Schraudolph-to-fp8);
    attnv uses fp8 DoubleRow with virtual K=256.
"""

import numpy as np
import ml_dtypes

from concourse import bacc, tile, mybir
from concourse import bass_utils
from concourse.bass import ds, ts
from concourse.masks import make_identity

F32 = mybir.dt.float32
BF16 = mybir.dt.bfloat16
I16 = mybir.dt.int16
F8 = mybir.dt.float8e4
I8 = mybir.dt.int8
DR = mybir.MatmulPerfMode.DoubleRow
EXP = mybir.ActivationFunctionType.Exp
IDENT = mybir.ActivationFunctionType.Identity
MULT = mybir.AluOpType.mult
ADD = mybir.AluOpType.add

B = 8
C = 512
HID = 64
N = 4096
NMT = N // 128      # 32 key tiles
QC = 512            # query chunk (PSUM bank width in f32)
PAIRS = NMT // 2    # 16 key-tile pairs
NSC = 4             # super-chunks of 2 query chunks

# Schraudolph exp->fp8e4m3 bit trick: i8 = s*A + Bc, bitcast to fp8
SCH_A = 8.0 / float(np.log(2.0))
SCH_B = 56.0 - 0.46

# (p, j) pairs whose exp runs on VectorE (12 of 32 per super-chunk; ScalarE
# also carries the qq-bias and the per-super-chunk O staging copies)
VEC_PJ = frozenset((p, j) for p in range(PAIRS) for j in range(2)
                   if (2 * p + j) % 32 in (1, 4, 7, 10, 13, 14, 16, 19, 22, 25,
                                           28, 31))


def build_bass(stage=4):
    nc = bacc.Bacc(
        "TRN2",
        target_bir_lowering=False,
        debug=False,
        enable_asserts=False,
        num_devices=B,
    )
    x = nc.dram_tensor("x", [C, N], BF16, kind="ExternalInput").ap()
    wkvT = nc.dram_tensor("wkvT", [C, 128], BF16, kind="ExternalInput").ap()
    wqqT = nc.dram_tensor("wqqT", [C, 128], BF16, kind="ExternalInput").ap()
    bkv = nc.dram_tensor("bkv", [128, 1], F32, kind="ExternalInput").ap()
    bqq = nc.dram_tensor("bqq", [128, 1], F32, kind="ExternalInput").ap()
    woT = nc.dram_tensor("woT", [HID, C], BF16, kind="ExternalInput").ap()
    y = nc.dram_tensor("y", [C, N], BF16, kind="ExternalOutput").ap()

    xr = x.rearrange("(a p) n -> p a n", p=128)   # [128, 4, N] channel = a*128+p
    yr = y.rearrange("(a p) n -> p a n", p=128)

    with tile.TileContext(nc) as tc:
        with (
            nc.allow_low_precision(reason="bf16/approx-exp attention is intended"),
            tc.tile_pool(name="const", bufs=1) as cpool,
            tc.tile_pool(name="big", bufs=1) as bigpool,
            tc.tile_pool(name="u", bufs=6) as upool,
            tc.tile_pool(name="work", bufs=2) as wpool,
            tc.tile_pool(name="yout", bufs=6) as ypool,
            tc.tile_pool(name="psum", bufs=3, space="PSUM") as pp,
        ):
            # ---- persistent tensors (allocate before constants so DMAs can
            # be emitted first) ----
            xb = bigpool.tile([128, 4, N], BF16)      # x (already bf16 in DRAM)
            vk = bigpool.tile([128, N], BF16)         # rows 0:64 v, 64:128 k
            klo = bigpool.tile([128, N], BF16)        # rows 0:64 = k
            qq = bigpool.tile([128, N], BF16)         # q*scale on both halves
            vt = bigpool.tile([128, NMT, 80], F8)     # v^T tiles + ones col 64
            Ob = bigpool.tile([128, N], BF16)         # rows 0:64 normalized O

            wkv = cpool.tile([128, 4, 128], BF16)
            wqq = cpool.tile([128, 4, 128], BF16)
            bkv_sb = cpool.tile([128, 1], F32)
            bqq_sb = cpool.tile([128, 1], F32)
            # w_out^T on both partition halves: the out-projection runs as
            # two concurrent row-tiled K=64 matmuls (the b_out bias is
            # folded into x on the host; QKV biases are compensated there)
            wo2 = cpool.tile([128, C], BF16)

            # critical DMA wave: x block 0 (one chunk per queue; SWDGE takes
            # two) plus the qkv weights; block-0 chunks lead each queue so
            # the first projection can start as early as possible
            def emit_x(b):
                cols = ts(b, 1024)
                nc.sync.dma_start(xb[:, 0, cols], xr[:, 0, cols])
                nc.scalar.dma_start(xb[:, 1, cols], xr[:, 1, cols])
                nc.gpsimd.dma_start(xb[:, 2, cols], xr[:, 2, cols])
                nc.gpsimd.dma_start(xb[:, 3, cols], xr[:, 3, cols])

            nc.scalar.dma_start(wkv[:, :, :], wkvT.rearrange("(a p) m -> p a m", p=128))
            # identity first on gpsimd: it feeds the PE warm-up matmuls
            ident_f = cpool.tile([64, 64], F32)
            make_identity(nc, ident_f[:, :])
            emit_x(0)
            nc.sync.dma_start(bkv_sb[:, :], bkv)
            nc.scalar.dma_start(wqq[:, :, :], wqqT.rearrange("(a p) m -> p a m", p=128))
            nc.scalar.dma_start(bqq_sb[:, :], bqq)
            nc.sync.dma_start(wo2[0:HID, :], woT)
            nc.sync.dma_start(wo2[HID:128, :], woT)

            # ---- constants (gpsimd/vector work after the DMA issues) ----
            ident = cpool.tile([64, 64], BF16)
            nc.vector.tensor_copy(ident[:, :], ident_f[:, :])
            wrm = cpool.tile([64, 512], BF16)
            nc.gpsimd.memset(wrm[:, :], 1.0)
            ones_f = cpool.tile([128, NMT], F32)
            nc.gpsimd.memset(ones_f[:, :], 1.0)

            nc.vector.tensor_copy(vt[:, :, HID], ones_f[:, 0:NMT])

            # HAM warm-up: a dense stream of N=512 matmuls (identity weights
            # against a memset tile -- the product is never read) keeps the
            # PE activity window busy while x streams in, so the projection
            # and score matmuls run at 2.4 GHz from the start.
            ps_w = pp.tile([64, 512], F32, tag="pair", name="ps_warm")
            for r in range(20):
                nc.tensor.matmul(
                    ps_w[:, :], ident[:, :], wrm[:, :],
                    start=(r == 0), stop=(r == 19),
                )

            def emit_vk_mms(b):
                nblk = ts(b, 1024)
                ps_vk = pp.tile([128, 1024], F32, tag="pair", name=f"psvk_{b}")
                for c2 in range(0, 1024, 512):
                    cols = ds(b * 1024 + c2, 512)
                    for kc in range(4):
                        nc.tensor.matmul(
                            ps_vk[:, c2:c2 + 512], wkv[:, kc, :], xb[:, kc, cols],
                            start=(kc == 0), stop=(kc == 3),
                        )
                return ps_vk

            def emit_vk_post(b, ps_vk):
                """bias add, k-low swap, v^T tiles for block b"""
                nblk = ts(b, 1024)
                nc.vector.tensor_scalar_add(vk[:, nblk], ps_vk[:, :], bkv_sb[:, 0:1])
                nc.sync.dma_start(klo[0:64, nblk], vk[64:128, nblk])
                for t2 in range(4):
                    mt = b * 8 + 2 * t2
                    ps_t = pp.tile([128, 128], BF16, tag="pair", bufs=3,
                                   name=f"pst_{mt}")
                    nc.tensor.transpose(ps_t[:, 0:64], vk[0:64, ts(mt, 128)],
                                        ident[:, :])
                    nc.tensor.transpose(ps_t[:, 64:128], vk[0:64, ts(mt + 1, 128)],
                                        ident[:, :])
                    src = ps_t.rearrange("p (two f) -> p two f", two=2)
                    if t2 % 2 == 0:
                        nc.scalar.copy(vt[:, mt:mt + 2, 0:HID], src[:, :, :])
                    else:
                        nc.vector.tensor_copy(vt[:, mt:mt + 2, 0:HID], src[:, :, :])

            def emit_proj_vk(b):
                emit_vk_post(b, emit_vk_mms(b))

            def emit_proj_qq(b):
                nblk = ts(b, 1024)
                ps_qq = pp.tile([128, 1024], F32, tag="pair", name=f"psqq_{b}")
                for c2 in range(0, 1024, 512):
                    cols = ds(b * 1024 + c2, 512)
                    for kc in range(4):
                        nc.tensor.matmul(
                            ps_qq[:, c2:c2 + 512], wqq[:, kc, :], xb[:, kc, cols],
                            start=(kc == 0), stop=(kc == 3),
                        )
                nc.scalar.activation(qq[:, nblk], ps_qq[:, :], IDENT,
                                     bias=bqq_sb[:, 0:1])

            # block 0: vk and qq matmuls back-to-back keep the PE stream
            # dense (no HAM dip); bias/transposes follow, then wave 2 of x
            ps_vk0 = emit_vk_mms(0)
            emit_proj_qq(0)
            emit_vk_post(0, ps_vk0)
            emit_x(1)

            ps_o_tiles = {}
            rb_tiles = {}
            otmp_tiles = {}

            def score_pair(qc, p):
                """scores for key pair p against query chunk qc -> PSUM tile"""
                qblk = ds(qc * QC, QC)
                mt0, mt1 = 2 * p, 2 * p + 1
                pr = pp.tile([128, 1024], F32, tag="pair", name=f"ps_{qc}_{p}")
                nc.tensor.matmul(
                    pr[:, 0:512], klo[0:64, ts(mt0, 128)], qq[0:64, qblk],
                    start=True, stop=True, tile_position=(0, 0),
                )
                nc.tensor.matmul(
                    pr[:, 512:1024], vk[64:128, ts(mt1, 128)], qq[64:128, qblk],
                    start=True, stop=True, tile_position=(64, 0),
                )
                return pr

            def exp_pair(qc, p, j, pr):
                u = upool.tile([128, 1024], F8, tag="u", name=f"u_{qc}_{p}")
                if (p, j) in VEC_PJ:
                    nc.vector.tensor_scalar(
                        u.bitcast(I8)[:, :], pr[:, :], SCH_A, SCH_B, MULT, ADD,
                    )
                else:
                    nc.scalar.activation(u[:, :], pr[:, :], EXP)
                return u

            def attnv_j(p, jslot, u, start, stop):
                # fp8 DoubleRow: one matmul contracts both key tiles of the
                # pair (virtual K=256); lhsT [128,2,65], rhs [128,2,512]
                nc.tensor.matmul(
                    ps_o_tiles_j[jslot][0:HID + 1, :],
                    vt[:, 2 * p:2 * p + 2, 0:HID + 1],
                    u.rearrange("p (two f) -> p two f", two=2),
                    start=start, stop=stop,
                    perf_mode=DR,
                )

            def emit_d_hops(qc, early_copy=True):
                """one [65,512] DVE copy stages O rows + denominator row to
                SBUF (same cost as copying the denominator row alone), so
                the PSUM accumulator slot frees after a single op and the
                reciprocal chain runs entirely off the critical path."""
                ps_o = ps_o_tiles[qc]
                dsb = wpool.tile([1, QC], F32, tag="d", name=f"dsb_{qc}")
                nc.vector.tensor_copy(dsb[:, :], ps_o[HID:HID + 1, :])
                if early_copy:
                    ot = wpool.tile([HID, QC], F32, tag="ot", name=f"ot_{qc}")
                    nc.scalar.copy(ot[:, :], ps_o[0:HID, :])
                    otmp_tiles[qc] = ot
                rsb = wpool.tile([1, QC], F32, tag="r", name=f"rsb_{qc}")
                nc.vector.reciprocal_approx_fast(rsb[:, :], dsb[:, :])
                rb = wpool.tile([HID, QC], F32, tag="rb", name=f"rb_{qc}")
                nc.gpsimd.partition_broadcast(rb[:, :], rsb[:, :], channels=HID)
                rb_tiles[qc] = rb

            def emit_norm(qc):
                qblk = ds(qc * QC, QC)
                ps_o = ps_o_tiles.pop(qc)
                if qc in otmp_tiles:
                    src = otmp_tiles.pop(qc)[:, :]
                else:
                    src = ps_o[0:HID, :]
                nc.vector.tensor_mul(Ob[0:HID, qblk], src,
                                     rb_tiles.pop(qc)[:, :])
                # duplicate to the upper partition half for the row-tiled
                # out-projection pair (scalar HWDGE queue: idle in steady
                # state, so the copy lands well before the h64 matmuls)
                nc.scalar.dma_start(Ob[HID:128, qblk], Ob[0:HID, qblk])

            def emit_outproj(qc, ocp):
                """out-projection for channel groups 2*ocp, 2*ocp+1 as two
                concurrent row-tiled K=64 matmuls"""
                qblk = ds(qc * QC, QC)
                for half in (0, 1):
                    oc = 2 * ocp + half
                    ps_y = pp.tile([128, QC], F32, tag="pair", bufs=3,
                                   name=f"psy_{qc}_{oc}")
                    if half == 0:
                        nc.tensor.matmul(
                            ps_y[:, :], wo2[0:HID, ts(oc, 128)], Ob[0:HID, qblk],
                            start=True, stop=True, tile_position=(0, 0),
                        )
                    else:
                        nc.tensor.matmul(
                            ps_y[:, :], wo2[HID:128, ts(oc, 128)],
                            Ob[HID:128, qblk],
                            start=True, stop=True, tile_position=(64, 0),
                        )
                    y_sb = ypool.tile([128, QC], BF16, tag="ysb",
                                      name=f"ysb_{qc}_{oc}")
                    nc.vector.tensor_add(y_sb[:, :], ps_y[:, :], xb[:, oc, qblk])
                    nc.sync.dma_start(yr[:, oc, qblk], y_sb[:, :])

            ps_o_tiles_j = {}

            def super_chunk(sc):
                qa, qb = 2 * sc, 2 * sc + 1
                for j, qc in enumerate((qa, qb)):
                    t = pp.tile([128, QC], F32, tag="o", bufs=2, name=f"pso_{qc}")
                    ps_o_tiles_j[j] = t
                    ps_o_tiles[qc] = t
                us = {}
                for p in range(PAIRS):
                    if sc == 0 and p in (4, 8, 12):
                        emit_proj_vk(p // 4)
                        if p < 12:
                            emit_x(p // 4 + 1)
                    if sc > 0 and p == 1:
                        emit_norm(qa - 2)
                        emit_norm(qb - 2)
                    for j, qc in enumerate((qa, qb)):
                        pr = score_pair(qc, p)
                        us[(p, j)] = exp_pair(qc, p, j, pr)
                    # attnv runs two pairs behind so exp latency stays off
                    # the PE critical path; out-proj of the previous
                    # super-chunk rides in the same untiled-mode window.
                    if p >= 2:
                        for j in (0, 1):
                            attnv_j(p - 2, j, us.pop((p - 2, j)),
                                    start=(p - 2 == 0), stop=False)
                        if sc > 0 and p in (5, 7, 9, 11):
                            emit_outproj(qa - 2 if p <= 7 else qb - 2,
                                         (p // 2) % 2)
                    if sc < NSC - 1 and p == 12:
                        emit_proj_qq(sc + 1)
                for j in (0, 1):
                    attnv_j(PAIRS - 2, j, us.pop((PAIRS - 2, j)),
                            start=False, stop=False)
                for j, qc in enumerate((qa, qb)):
                    attnv_j(PAIRS - 1, j, us.pop((PAIRS - 1, j)),
                            start=False, stop=True)
                    emit_d_hops(qc, early_copy=True)

            def super_chunk_split(sc):
                """last super-chunk: qa fully, then qb; qa's normalize and
                out-projection overlap qb's attention phase."""
                qa, qb = 2 * sc, 2 * sc + 1
                for j, qc in enumerate((qa, qb)):
                    t = pp.tile([128, QC], F32, tag="o", bufs=2, name=f"pso_{qc}")
                    ps_o_tiles_j[j] = t
                    ps_o_tiles[qc] = t
                us = {}
                # ---- phase 0: all of qa's pairs ----
                for p in range(PAIRS):
                    if p == 1:
                        emit_norm(qa - 2)
                        emit_norm(qb - 2)
                    pr = score_pair(qa, p)
                    us[p] = exp_pair(qa, p, 0, pr)
                    if p >= 2:
                        attnv_j(p - 2, 0, us.pop(p - 2),
                                start=(p - 2 == 0), stop=False)
                        if p in (5, 7, 9, 11):
                            emit_outproj(qa - 2 if p <= 7 else qb - 2,
                                         (p // 2) % 2)
                # transition: qb's first four score pairs (full u-ring
                # depth) cover qa's last attnv exp latency and the
                # denominator chain
                usb = {}
                pr = score_pair(qb, 0)
                usb[0] = exp_pair(qb, 0, 1, pr)
                attnv_j(PAIRS - 2, 0, us.pop(PAIRS - 2), start=False, stop=False)
                pr = score_pair(qb, 1)
                usb[1] = exp_pair(qb, 1, 1, pr)
                attnv_j(PAIRS - 1, 0, us.pop(PAIRS - 1), start=False, stop=True)
                emit_d_hops(qa)
                pr = score_pair(qb, 2)
                usb[2] = exp_pair(qb, 2, 1, pr)
                attnv_j(0, 1, usb.pop(0), start=True, stop=False)
                pr = score_pair(qb, 3)
                usb[3] = exp_pair(qb, 3, 1, pr)
                attnv_j(1, 1, usb.pop(1), start=False, stop=False)
                emit_norm(qa)
                # ---- phase 1: qb's remaining pairs; qa's out-projection
                # rides inside ----
                for p in range(4, PAIRS):
                    pr = score_pair(qb, p)
                    usb[p] = exp_pair(qb, p, 1, pr)
                    attnv_j(p - 2, 1, usb.pop(p - 2), start=False, stop=False)
                    if p in (5, 13):
                        emit_outproj(qa, {5: 0, 13: 1}[p])
                attnv_j(PAIRS - 2, 1, usb.pop(PAIRS - 2), start=False, stop=False)
                attnv_j(PAIRS - 1, 1, usb.pop(PAIRS - 1), start=False, stop=True)
                emit_d_hops(qb)
                emit_norm(qb)
                for ocp in range(2):
                    emit_outproj(qb, ocp)

            for sc in range(NSC - 1):
                super_chunk(sc)
            super_chunk_split(NSC - 1)

    nc.compile()
    return nc


_NC = None
_NC_STAGE = None


def _get_nc(stage=4):
    global _NC, _NC_STAGE
    if _NC is None or _NC_STAGE != stage:
        _NC = build_bass(stage)
        _NC_STAGE = stage
    return _NC


def make_in_maps(x, w_in, b_in, w_out, b_out):
    scale = 1.0 / np.sqrt(np.float32(HID))
    w = np.asarray(w_in, np.float32)
    b = np.asarray(b_in, np.float32)
    bo = np.asarray(b_out, np.float32)                     # [512]
    # fold b_out into x (the skip connection carries it) and compensate
    # the qkv biases: q = Wq (x+bo) + (bq - Wq bo) == Wq x + bq, exactly
    bq = (b[0:HID] - w[0:HID] @ bo) * scale
    bk = b[HID:2 * HID] - w[HID:2 * HID] @ bo
    bv = b[2 * HID:] - w[2 * HID:] @ bo
    wq = np.ascontiguousarray(w[0:HID].T) * scale          # [512, 64]
    wk = np.ascontiguousarray(w[HID:2 * HID].T)
    wv = np.ascontiguousarray(w[2 * HID:3 * HID].T)
    wkvT = np.concatenate([wv, wk], axis=1)                # [512, 128] = [v|k]
    wqqT = np.concatenate([wq, wq], axis=1)
    bkv = np.concatenate([bv, bk]).reshape(128, 1)
    bqq = np.concatenate([bq, bq]).reshape(128, 1)
    woT = np.ascontiguousarray(np.asarray(w_out, np.float32).T)  # [64, 512]
    x = np.asarray(x, np.float32) + bo[None, :, None, None]
    return [
        {
            "x": np.ascontiguousarray(
                x[bb].reshape(C, N).astype(ml_dtypes.bfloat16)),
            "wkvT": np.ascontiguousarray(wkvT.astype(ml_dtypes.bfloat16)),
            "wqqT": np.ascontiguousarray(wqqT.astype(ml_dtypes.bfloat16)),
            "bkv": np.ascontiguousarray(bkv, np.float32),
            "bqq": np.ascontiguousarray(bqq, np.float32),
            "woT": np.ascontiguousarray(woT.astype(ml_dtypes.bfloat16)),
        }
        for bb in range(B)
    ]


def kernel(x, w_in, b_in, w_out, b_out):
    nc = _get_nc()
    in_maps = make_in_maps(x, w_in, b_in, w_out, b_out)
    res = bass_utils.run_bass_kernel_spmd(nc, in_maps, core_ids=list(range(B)))
    H = int(np.sqrt(N))
    out = np.stack([
        np.asarray(res.results[bb]["y"]).astype(np.float32).reshape(C, H, H)
        for bb in range(B)
    ])
    return out
